# revision 1
# baseline (speedup 1.0000x reference)
"""ANP-MR ShapeNet1D forward pass, data-parallel over the task dimension
across 8 Trainium2 NeuronCores.

Sharding: T=16 tasks -> 8 devices x 2 tasks. Each device encodes its own
context/query images (conv encoder), runs the Performer attention locally,
and decodes. All parameters are replicated (broadcast) to every core; the
only sharded tensors are the three task-indexed inputs. Output is gathered
back to the full (16, 32, 2) shape on the host.
"""

import functools

import jax
import jax.numpy as jnp
import numpy as np

# Problem shapes (hardcoded per the spec; kernel.py must be self-contained).
T, NC, NQ = 16, 32, 32
H, W, C = 128, 128, 1
DW = 256
NDEV = 8

_SHARDED = ("batch_train_images", "label_train", "batch_test_images")


def _conv(x, w, b, stride):
    y = jax.lax.conv_general_dilated(
        x, w, (stride, stride), [(1, 1), (1, 1)],
        dimension_numbers=("NCHW", "OIHW", "NCHW"))
    return y + b[None, :, None, None]


def _maxpool2(x):
    return jax.lax.reduce_window(
        x, -jnp.inf, jax.lax.max, (1, 1, 2, 2), (1, 1, 2, 2), "VALID")


def _encode(x, c1w, c1b, c2w, c2b, c3w, c3b, lw, lb):
    x = jax.nn.relu(_conv(x, c1w, c1b, 2))
    x = jax.nn.relu(_conv(x, c2w, c2b, 2))
    x = _maxpool2(x)
    x = jax.nn.relu(_conv(x, c3w, c3b, 2))
    x = x.reshape(x.shape[0], -1)
    return x @ lw.T + lb


def _softmax_kernel(x, proj, is_query):
    dn = x.shape[-1] ** -0.25
    ratio = proj.shape[0] ** -0.5
    xn = x * dn
    xd = jnp.einsum("bhnd,fd->bhnf", xn, proj)
    diag = 0.5 * jnp.sum(xn * xn, axis=-1, keepdims=True)
    if is_query:
        m = jnp.max(xd, axis=-1, keepdims=True)
    else:
        m = jnp.max(xd, axis=(-1, -2), keepdims=True)
    return ratio * (jnp.exp(xd - diag - jax.lax.stop_gradient(m)) + 1e-4)


def _forward(batch_train_images, label_train, batch_test_images,
             conv1_w, conv1_b, conv2_w, conv2_b, conv3_w, conv3_b,
             enc_lw, enc_lb, ty_w, ty_b, er_w1, er_b1, er_w2, er_b2,
             er_w3, er_b3, wq, wq_b, wk, wk_b, wv, wv_b, wo_w, wo_b,
             rz_w, rz_b, d0_w1, d0_b1, d0_w2, d0_b2, d0_w3, d0_b3, proj):
    # Per-device shapes: images are (Tloc, N, C, H, W) with Tloc = T // NDEV.
    tloc = batch_train_images.shape[0]
    enc = lambda im: _encode(im, conv1_w, conv1_b, conv2_w, conv2_b,
                             conv3_w, conv3_b, enc_lw, enc_lb)
    x_qry = enc(batch_test_images.reshape(-1, C, H, W)).reshape(tloc, NQ, DW)
    x_ctx = enc(batch_train_images.reshape(-1, C, H, W)).reshape(tloc, NC, DW)
    ly = label_train @ ty_w.T + ty_b
    x = jnp.concatenate([x_ctx, ly], axis=2)
    h = jax.nn.relu(x @ er_w1.T + er_b1)
    h = jax.nn.relu(h @ er_w2.T + er_b2)
    rs = h @ er_w3.T + er_b3
    k_all = jnp.einsum("bnd,hed->bhne", x_ctx, wk) + wk_b[None, :, None, :]
    v_all = jnp.einsum("bnd,hed->bhne", rs, wv) + wv_b[None, :, None, :]
    q_all = jnp.einsum("bnd,hed->bhne", x_qry, wq) + wq_b[None, :, None, :]
    qp = _softmax_kernel(q_all, proj, True)
    kp = _softmax_kernel(k_all, proj, False)
    d_inv = 1.0 / jnp.einsum("bhnf,bhf->bhn", qp, kp.sum(axis=2))
    ctx_mat = jnp.einsum("bhnf,bhne->bhfe", kp, v_all)
    outs = jnp.einsum("bhnf,bhfe->bhne", qp, ctx_mat) * d_inv[..., None]
    outs = outs.transpose(0, 2, 3, 1).reshape(tloc, NQ, -1)
    rep = outs @ wo_w.T + wo_b
    zz = rep @ rz_w.T + rz_b
    xz = jnp.concatenate([x_qry, zz], axis=-1)
    h = jax.nn.relu(xz @ d0_w1.T + d0_b1)
    h = jax.nn.relu(h @ d0_w2.T + d0_b2)
    return jnp.tanh(h @ d0_w3.T + d0_b3)


@functools.lru_cache(maxsize=1)
def _compiled():
    devs = jax.devices()[:NDEV]
    in_axes = ({k: (0 if k in _SHARDED else None) for k in _ARG_ORDER},)

    def fn(kw):
        return _forward(**kw)

    return jax.pmap(fn, in_axes=in_axes, devices=devs)


_ARG_ORDER = (
    "batch_train_images", "label_train", "batch_test_images",
    "conv1_w", "conv1_b", "conv2_w", "conv2_b", "conv3_w", "conv3_b",
    "enc_lw", "enc_lb", "ty_w", "ty_b", "er_w1", "er_b1", "er_w2", "er_b2",
    "er_w3", "er_b3", "wq", "wq_b", "wk", "wk_b", "wv", "wv_b",
    "wo_w", "wo_b", "rz_w", "rz_b",
    "d0_w1", "d0_b1", "d0_w2", "d0_b2", "d0_w3", "d0_b3", "proj",
)


def kernel(**inputs):
    kw = {}
    for name in _ARG_ORDER:
        v = np.asarray(inputs[name])
        if name in _SHARDED:
            # (T, ...) -> (NDEV, T/NDEV, ...)
            v = v.reshape((NDEV, T // NDEV) + v.shape[1:])
        kw[name] = v
    out = _compiled()(kw)  # (NDEV, Tloc, NQ, 2)
    pr_y_mu = np.asarray(out, dtype=np.float32).reshape(T, NQ, 2)
    kl = np.float32(0.0)
    return (pr_y_mu, kl)
